# revision 21
# baseline (speedup 1.0000x reference)
"""CAMIL self-attention Trainium2 kernel (8 NeuronCores, SPMD).

Reference computation (B=2, N=8192, IN_DIM=ATT_DIM=512):
    q = X @ Wq ; k = X @ Wk ; v = X @ Wv
    w_i = inv_scale * m_i * sum_d q[i,d] * (adj @ (k*m))[i,d]
    L   = softmax(w, axis=bag)[:, :, None] * v

Sharding: 8 cores = (batch 2) x (4 row-blocks of 2048). Each core holds
adj[b, i_block, :]^T (pre-transposed on host so the contraction dim j lands
on SBUF partitions), computes its w-slice on-device, the 8 cores AllGather
w (2 KB/core), every core computes the softmax normalizers for both batches
locally, then scales its own v rows.

The big matmul (adj^T-tile stationary, k_m moving, fp32r) runs at 1 cyc/row,
so the kernel sits near the HBM roofline for streaming the 512 MB adj.
v is computed with plain fp32 matmuls (it multiplies the near-one-hot softmax
directly, so it carries the output's absmax precision).
"""

import numpy as np
from contextlib import ExitStack

import concourse.bass as bass
import concourse.bacc as bacc
import concourse.tile as tile
from concourse import mybir, bass_isa
from concourse.bass_utils import run_bass_kernel_spmd

F32 = mybir.dt.float32
F32R = mybir.dt.float32r
BF16 = mybir.dt.bfloat16

B, N, D = 2, 8192, 512
RPC = N // 4  # rows per core: 2048
INV_SCALE = float(1.0 / np.sqrt(np.float32(D)))

_CACHE = {}


def _build(stage="full"):
    # stage: debug gate — "p1" (projections only), "p2" (+w), "coll" (+AllGather),
    # "full" (everything). kernel() always uses "full".
    nc = bacc.Bacc(None, target_bir_lowering=False, debug=False, num_devices=8)

    adjt = nc.dram_tensor("adjt", [N, RPC], BF16, kind="ExternalInput")
    xt = nc.dram_tensor("xt", [D, N], BF16, kind="ExternalInput")
    xtq = nc.dram_tensor("xtq", [D, RPC], F32, kind="ExternalInput")
    xtqb = nc.dram_tensor("xtqb", [D, RPC], BF16, kind="ExternalInput")
    wqd = nc.dram_tensor("wq", [D, D], BF16, kind="ExternalInput")
    wkd = nc.dram_tensor("wk", [D, D], BF16, kind="ExternalInput")
    wvd = nc.dram_tensor("wv", [D, D], F32, kind="ExternalInput")
    maskqd = nc.dram_tensor("maskq", [128, 64], F32, kind="ExternalInput")
    maskownd = nc.dram_tensor("maskown", [128, 16], F32, kind="ExternalInput")
    bseld = nc.dram_tensor("bsel", [128, 2], F32, kind="ExternalInput")
    loutd = nc.dram_tensor("lout", [RPC, D], F32, kind="ExternalOutput")

    w_locd = nc.dram_tensor("w_loc", [128, 16], F32)
    w_alld = nc.dram_tensor("w_all", [8, 128, 16], F32, addr_space="Shared")

    MUL = mybir.AluOpType.mult
    ADD = mybir.AluOpType.add
    X_AX = mybir.AxisListType.X

    with tile.TileContext(nc) as tc, ExitStack() as ctx:
        wtail = ctx.enter_context(tc.tile_pool(name="wtail", bufs=1))
        bigctx = ExitStack()
        big = bigctx.enter_context(tc.tile_pool(name="big", bufs=1))

        km_s = big.tile([128, 64, D], BF16)      # k*mask, [j-part, j-chunk, d]
        q_s = big.tile([128, 16, D], F32)        # own q rows
        wq_s = big.tile([128, 4, D], BF16)
        wk_s = big.tile([128, 4, D], BF16)
        maskq_s = big.tile([128, 64], F32)
        wv_s = big.tile([128, 4, D], F32R)

        w_sb = wtail.tile([128, 16], F32)        # own w (pre-mask)
        w2 = wtail.tile([128, 16], F32)          # own w (masked)
        maskown_s = wtail.tile([128, 16], F32)
        v_sb = wtail.tile([128, 16, D], F32)     # own v rows (fp32r matmul)
        bsel_s = wtail.tile([128, 2], F32)

        nc.sync.dma_start(wq_s[:], wqd[:].rearrange("(cc p) d -> p cc d", p=128))
        nc.sync.dma_start(wk_s[:], wkd[:].rearrange("(cc p) d -> p cc d", p=128))
        nc.sync.dma_start(maskq_s[:], maskqd[:])
        nc.sync.dma_start(wv_s[:], wvd[:].bitcast(F32R).rearrange("(cc p) d -> p cc d", p=128))
        nc.sync.dma_start(maskown_s[:], maskownd[:])
        nc.sync.dma_start(bsel_s[:], bseld[:])

        # ---- Phase 1: k_m (all N rows) and q (own rows) projections -------
        with (
            tc.tile_pool(name="p1", bufs=8) as p1pool,
            tc.tile_pool(name="ps1", bufs=4, space="PSUM") as ps1,
        ):
            for jp in range(16):  # panels of 512 bag rows
                xtiles = []
                for cc in range(4):
                    xt_t = p1pool.tile([128, 512], BF16, tag="xtp")
                    nc.sync.dma_start(
                        xt_t[:],
                        xt[cc * 128:(cc + 1) * 128, jp * 512:(jp + 1) * 512],
                    )
                    xtiles.append(xt_t)
                for jc2 in range(4):
                    jc = jp * 4 + jc2
                    ps_k = ps1.tile([128, D], F32, tag="psk")
                    for cc in range(4):
                        nc.tensor.matmul(
                            ps_k[:],
                            lhsT=xtiles[cc][:, jc2 * 128:(jc2 + 1) * 128],
                            rhs=wk_s[:, cc, :],
                            start=(cc == 0),
                            stop=(cc == 3),
                        )
                    nc.vector.tensor_scalar_mul(km_s[:, jc, :], ps_k[:], maskq_s[:, jc:jc + 1])
            for gp in range(4):  # own panels of 512 rows
                xqt = []
                for cc in range(4):
                    xq_t = p1pool.tile([128, 512], BF16, tag="xtp")
                    nc.sync.dma_start(
                        xq_t[:],
                        xtqb[cc * 128:(cc + 1) * 128, gp * 512:(gp + 1) * 512],
                    )
                    xqt.append(xq_t)
                xvt = []
                for cc in range(4):
                    xv_t = p1pool.tile([128, 512], F32R, tag="xvp")
                    nc.sync.dma_start(
                        xv_t[:],
                        xtq[cc * 128:(cc + 1) * 128, gp * 512:(gp + 1) * 512].bitcast(F32R),
                    )
                    xvt.append(xv_t)
                for t2 in range(4):
                    t = gp * 4 + t2
                    ps_q = ps1.tile([128, D], F32, tag="psk")
                    for cc in range(4):
                        nc.tensor.matmul(
                            ps_q[:],
                            lhsT=xqt[cc][:, t2 * 128:(t2 + 1) * 128],
                            rhs=wq_s[:, cc, :],
                            start=(cc == 0),
                            stop=(cc == 3),
                        )
                    nc.vector.tensor_copy(q_s[:, t, :], ps_q[:])
                    ps_v = ps1.tile([128, D], F32, tag="psk")
                    for cc in range(4):
                        nc.tensor.matmul(
                            ps_v[:],
                            lhsT=xvt[cc][:, t2 * 128:(t2 + 1) * 128],
                            rhs=wv_s[:, cc, :],
                            start=(cc == 0),
                            stop=(cc == 3),
                        )
                    nc.vector.tensor_copy(v_sb[:, t, :], ps_v[:])

        if stage == "p1":
            with tc.tile_pool(name="dbg", bufs=2) as dbg:
                for t in range(16):
                    dt_ = dbg.tile([128, D], F32, tag="dbg")
                    nc.vector.tensor_copy(dt_[:], km_s[:, t, :])
                    nc.sync.dma_start(loutd[t * 128:(t + 1) * 128, :], dt_[:])
            bigctx.close()

        # ---- Phase 2: agg = adj_block @ k_m ; w = inv_scale * rowdot(q, agg)
        if stage != "p1":
            with (
                tc.tile_pool(name="s2", bufs=12) as s2pool,
                tc.tile_pool(name="scrp", bufs=2) as scrpool,
                tc.tile_pool(name="ps2", bufs=8, space="PSUM") as ps2,
            ):
                for qq in range(4):  # quarters of 512 own rows -> 4 PSUM banks
                    aggs = [ps2.tile([128, D], F32, tag="agg", name=f"agg_{qq}_{i}") for i in range(4)]
                    for jb in range(16):  # batches of 4 j-chunks (1 MB DMA)
                        at = s2pool.tile([128, 4, 512], BF16, tag="adjs")
                        nc.sync.dma_start(
                            at[:],
                            adjt[jb * 512:(jb + 1) * 512, qq * 512:(qq + 1) * 512]
                            .rearrange("(jc2 p) i -> p jc2 i", p=128),
                        )
                        for jc2 in range(4):
                            jc = jb * 4 + jc2
                            for is_ in range(4):
                                nc.tensor.matmul(
                                    aggs[is_][:],
                                    lhsT=at[:, jc2, is_ * 128:(is_ + 1) * 128],
                                    rhs=km_s[:, jc, :],
                                    start=(jc == 0),
                                    stop=(jc == 63),
                                )
                    for is_ in range(4):
                        t = qq * 4 + is_
                        # NOTE: tensor_tensor_reduce with a PSUM in0 faults the
                        # device (HW-only, sim-clean) — use mul + reduce instead.
                        scr = scrpool.tile([128, D], F32, tag="scr")
                        nc.vector.tensor_mul(scr[:], aggs[is_][:], q_s[:, t, :])
                        nc.vector.reduce_sum(out=w_sb[:, t:t + 1], in_=scr[:], axis=X_AX)

            bigctx.close()  # frees km/q/wq/wk (128+32+16 KB/partition) for the tail

            # ---- Phase 3: mask own w (and inv_scale), publish, AllGather --
            nc.vector.tensor_scalar_mul(w2[:], w_sb[:], INV_SCALE)
            nc.vector.tensor_mul(w2[:], w2[:], maskown_s[:])
            if stage == "p2":
                nc.sync.dma_start(loutd[0:128, 0:16], w2[:])
            else:
                nc.sync.dma_start(w_locd[:], w2[:])
                nc.gpsimd.collective_compute(
                    "AllGather",
                    mybir.AluOpType.bypass,
                    replica_groups=[[0, 1, 2, 3, 4, 5, 6, 7]],
                    ins=[w_locd[:]],
                    outs=[w_alld[:]],
                )

        if stage == "coll":
            with tc.tile_pool(name="dbg2", bufs=1) as dbg2:
                wdbg = dbg2.tile([128, 128], F32)
                nc.sync.dma_start(
                    wdbg[:].rearrange("p (g t) -> p g t", g=8),
                    w_alld[:].rearrange("g p t -> p g t"),
                )
                nc.sync.dma_start(loutd[0:128, 0:128], wdbg[:])

        if stage == "full":
            with (
                tc.tile_pool(name="tail", bufs=1) as tailp,
                tc.tile_pool(name="ltp", bufs=3) as ltp,
            ):
                # ---- Phase 4: softmax normalizers for both batches --------
                wall = tailp.tile([128, 128], F32)
                nc.sync.dma_start(
                    wall[:].rearrange("p (g t) -> p g t", g=8),
                    w_alld[:].rearrange("g p t -> p g t"),
                )
                gnegs, rinvs = [], []
                for h in range(2):
                    wh = wall[:, h * 64:(h + 1) * 64]
                    m1 = tailp.tile([128, 1], F32, tag=f"m1_{h}", name=f"m1_{h}")
                    nc.vector.reduce_max(out=m1[:], in_=wh, axis=X_AX)
                    gmax = tailp.tile([128, 1], F32, tag=f"gmax_{h}", name=f"gmax_{h}")
                    nc.gpsimd.partition_all_reduce(
                        gmax[:], m1[:], channels=128, reduce_op=bass_isa.ReduceOp.max
                    )
                    gneg = tailp.tile([128, 1], F32, tag=f"gneg_{h}", name=f"gneg_{h}")
                    nc.vector.tensor_scalar_mul(gneg[:], gmax[:], -1.0)
                    eh = tailp.tile([128, 64], F32, tag=f"eh_{h}", name=f"eh_{h}")
                    nc.scalar.activation(
                        out=eh[:], in_=wh, func=mybir.ActivationFunctionType.Exp,
                        bias=gneg[:], scale=1.0,
                    )
                    s1 = tailp.tile([128, 1], F32, tag=f"s1_{h}", name=f"s1_{h}")
                    nc.vector.reduce_sum(out=s1[:], in_=eh[:], axis=X_AX)
                    gsum = tailp.tile([128, 1], F32, tag=f"gsum_{h}", name=f"gsum_{h}")
                    nc.gpsimd.partition_all_reduce(
                        gsum[:], s1[:], channels=128, reduce_op=bass_isa.ReduceOp.add
                    )
                    rinv = tailp.tile([128, 1], F32, tag=f"rinv_{h}", name=f"rinv_{h}")
                    nc.vector.reciprocal(rinv[:], gsum[:])
                    gnegs.append(gneg)
                    rinvs.append(rinv)

                # select my batch's normalizers via bsel (per-core input)
                ga = tailp.tile([128, 1], F32, tag="ga")
                gb = tailp.tile([128, 1], F32, tag="gb")
                gneg_my = tailp.tile([128, 1], F32, tag="gneg_my")
                nc.vector.tensor_mul(ga[:], gnegs[0][:], bsel_s[:, 0:1])
                nc.vector.tensor_mul(gb[:], gnegs[1][:], bsel_s[:, 1:2])
                nc.vector.tensor_add(gneg_my[:], ga[:], gb[:])
                ra = tailp.tile([128, 1], F32, tag="ra")
                rb = tailp.tile([128, 1], F32, tag="rb")
                rinv_my = tailp.tile([128, 1], F32, tag="rinv_my")
                nc.vector.tensor_mul(ra[:], rinvs[0][:], bsel_s[:, 0:1])
                nc.vector.tensor_mul(rb[:], rinvs[1][:], bsel_s[:, 1:2])
                nc.vector.tensor_add(rinv_my[:], ra[:], rb[:])

                eown = tailp.tile([128, 16], F32, tag="eown")
                nc.scalar.activation(
                    out=eown[:], in_=w2[:], func=mybir.ActivationFunctionType.Exp,
                    bias=gneg_my[:], scale=1.0,
                )
                pown = tailp.tile([128, 16], F32, tag="pown")
                nc.vector.tensor_scalar_mul(pown[:], eown[:], rinv_my[:])

                # ---- Phase 5b: L rows = p_i * v_i -------------------------
                for t in range(16):
                    lt = ltp.tile([128, D], F32, tag="lt")
                    nc.vector.tensor_scalar_mul(lt[:], v_sb[:, t, :], pown[:, t:t + 1])
                    nc.sync.dma_start(loutd[t * 128:(t + 1) * 128, :], lt[:])

    nc.finalize()
    return nc


def _prep_inputs(X, adj, mask, Wqk, Wv):
    import ml_dtypes
    bf16 = ml_dtypes.bfloat16
    X = np.ascontiguousarray(np.asarray(X, dtype=np.float32))
    adj = np.asarray(adj, dtype=np.float32)
    mask = np.ascontiguousarray(np.asarray(mask, dtype=np.float32))
    Wqk = np.asarray(Wqk, dtype=np.float32)
    Wv = np.ascontiguousarray(np.asarray(Wv, dtype=np.float32))
    wq_h = np.ascontiguousarray(Wqk[:, :D].astype(bf16))
    wk_h = np.ascontiguousarray(Wqk[:, D:].astype(bf16))

    in_maps = []
    for b in range(B):
        xt_b = np.ascontiguousarray(X[b].T)
        xt_bh = np.ascontiguousarray(xt_b.astype(bf16))
        adjt_bh = np.ascontiguousarray(adj[b].astype(bf16).T)
        maskq_b = np.ascontiguousarray(mask[b].reshape(64, 128).T)
        for r in range(4):
            i0 = r * RPC
            bsel = np.zeros((128, 2), np.float32)
            bsel[:, b] = 1.0
            in_maps.append({
                "adjt": np.ascontiguousarray(adjt_bh[:, i0:i0 + RPC]),
                "xt": xt_bh,
                "xtq": np.ascontiguousarray(xt_b[:, i0:i0 + RPC]),
                "xtqb": np.ascontiguousarray(xt_bh[:, i0:i0 + RPC]),
                "wq": wq_h,
                "wk": wk_h,
                "wv": Wv,
                "maskq": maskq_b,
                "maskown": np.ascontiguousarray(mask[b, i0:i0 + RPC].reshape(16, 128).T),
                "bsel": bsel,
            })
    return in_maps


def _run(inputs, **kwargs):
    if "nc" not in _CACHE:
        _CACHE["nc"] = _build()
    nc = _CACHE["nc"]
    in_maps = _prep_inputs(**inputs)
    res = run_bass_kernel_spmd(nc, in_maps, list(range(8)), **kwargs)
    L = np.empty((B, N, D), np.float32)
    for c in range(8):
        b, r = divmod(c, 4)
        L[b, r * RPC:(r + 1) * RPC] = res.results[c]["lout"]
    return L, res


def kernel(X, adj, mask, Wqk, Wv):
    L, _ = _run(dict(X=X, adj=adj, mask=mask, Wqk=Wqk, Wv=Wv))
    return L


# revision 23
# speedup vs baseline: 1.3762x; 1.3762x over previous
"""CAMIL self-attention Trainium2 kernel (8 NeuronCores, SPMD).

Reference computation (B=2, N=8192, IN_DIM=ATT_DIM=512):
    q = X @ Wq ; k = X @ Wk ; v = X @ Wv
    w_i = inv_scale * m_i * sum_d q[i,d] * (adj @ (k*m))[i,d]
    L   = softmax(w, axis=bag)[:, :, None] * v

Sharding: 8 cores = (batch 2) x (4 row-blocks of 2048). Each core holds
adj[b, i_block, :]^T (pre-transposed on host so the contraction dim j lands
on SBUF partitions), computes its w-slice on-device, the 8 cores AllGather
w (2 KB/core), every core computes the softmax normalizers for both batches
locally, then scales its own v rows.

The big matmul (adj^T-tile stationary, k_m moving, fp32r) runs at 1 cyc/row,
so the kernel sits near the HBM roofline for streaming the 512 MB adj.
v is computed with plain fp32 matmuls (it multiplies the near-one-hot softmax
directly, so it carries the output's absmax precision).
"""

import numpy as np
from contextlib import ExitStack

import concourse.bass as bass
import concourse.bacc as bacc
import concourse.tile as tile
from concourse import mybir, bass_isa
from concourse.bass_utils import run_bass_kernel_spmd

F32 = mybir.dt.float32
F32R = mybir.dt.float32r
BF16 = mybir.dt.bfloat16
FP8 = mybir.dt.float8e4

B, N, D = 2, 8192, 512
RPC = N // 4  # rows per core: 2048
INV_SCALE = float(1.0 / np.sqrt(np.float32(D)))

_CACHE = {}


def _build(stage="full"):
    # stage: debug gate — "p1" (projections only), "p2" (+w), "coll" (+AllGather),
    # "full" (everything). kernel() always uses "full".
    nc = bacc.Bacc(None, target_bir_lowering=False, debug=False, num_devices=8)

    adjt = nc.dram_tensor("adjt", [N, RPC], FP8, kind="ExternalInput")
    xt = nc.dram_tensor("xt", [D, N], BF16, kind="ExternalInput")
    xtq = nc.dram_tensor("xtq", [D, RPC], F32, kind="ExternalInput")
    xtqb = nc.dram_tensor("xtqb", [D, RPC], BF16, kind="ExternalInput")
    wqd = nc.dram_tensor("wq", [D, D], BF16, kind="ExternalInput")
    wkd = nc.dram_tensor("wk", [D, D], BF16, kind="ExternalInput")
    wvd = nc.dram_tensor("wv", [D, D], F32, kind="ExternalInput")
    maskqd = nc.dram_tensor("maskq", [128, 64], F32, kind="ExternalInput")
    maskownd = nc.dram_tensor("maskown", [128, 16], F32, kind="ExternalInput")
    bseld = nc.dram_tensor("bsel", [128, 2], F32, kind="ExternalInput")
    loutd = nc.dram_tensor("lout", [RPC, D], F32, kind="ExternalOutput")

    w_locd = nc.dram_tensor("w_loc", [128, 16], F32)
    w_alld = nc.dram_tensor("w_all", [8, 128, 16], F32, addr_space="Shared")

    MUL = mybir.AluOpType.mult
    ADD = mybir.AluOpType.add
    X_AX = mybir.AxisListType.X

    with tile.TileContext(nc) as tc, ExitStack() as ctx:
        wtail = ctx.enter_context(tc.tile_pool(name="wtail", bufs=1))
        bigctx = ExitStack()
        big = bigctx.enter_context(tc.tile_pool(name="big", bufs=1))

        km_s = big.tile([128, 64, D], FP8)       # k*mask, [j-part, j-chunk, d]
        q_s = big.tile([128, 16, D], F32)        # own q rows
        wq_s = big.tile([128, 4, D], BF16)
        wk_s = big.tile([128, 4, D], BF16)
        maskq_s = big.tile([128, 64], F32)
        wv_s = big.tile([128, 4, D], F32R)

        w_sb = wtail.tile([128, 16], F32)        # own w (pre-mask)
        w2 = wtail.tile([128, 16], F32)          # own w (masked)
        maskown_s = wtail.tile([128, 16], F32)
        v_sb = wtail.tile([128, 16, D], F32)     # own v rows (fp32r matmul)
        bsel_s = wtail.tile([128, 2], F32)

        nc.sync.dma_start(wq_s[:], wqd[:].rearrange("(cc p) d -> p cc d", p=128))
        nc.sync.dma_start(wk_s[:], wkd[:].rearrange("(cc p) d -> p cc d", p=128))
        nc.sync.dma_start(maskq_s[:], maskqd[:])
        nc.sync.dma_start(wv_s[:], wvd[:].bitcast(F32R).rearrange("(cc p) d -> p cc d", p=128))
        nc.sync.dma_start(maskown_s[:], maskownd[:])
        nc.sync.dma_start(bsel_s[:], bseld[:])

        # ---- Phase 1: k_m (all N rows) and q (own rows) projections -------
        with (
            tc.tile_pool(name="p1", bufs=16) as p1pool,
            tc.tile_pool(name="ps1", bufs=6, space="PSUM") as ps1,
        ):
            for jp in range(16):  # panels of 512 bag rows
                xtiles = []
                for cc in range(4):
                    xt_t = p1pool.tile([128, 512], BF16, tag="xtp")
                    nc.sync.dma_start(
                        xt_t[:],
                        xt[cc * 128:(cc + 1) * 128, jp * 512:(jp + 1) * 512],
                    )
                    xtiles.append(xt_t)
                for jc2 in range(4):
                    jc = jp * 4 + jc2
                    ps_k = ps1.tile([128, D], F32, tag="psk")
                    for cc in range(4):
                        nc.tensor.matmul(
                            ps_k[:],
                            lhsT=xtiles[cc][:, jc2 * 128:(jc2 + 1) * 128],
                            rhs=wk_s[:, cc, :],
                            start=(cc == 0),
                            stop=(cc == 3),
                        )
                    nc.vector.tensor_scalar_mul(km_s[:, jc, :], ps_k[:], maskq_s[:, jc:jc + 1])
            for gp in range(4):  # own panels of 512 rows
                xqt = []
                for cc in range(4):
                    xq_t = p1pool.tile([128, 512], BF16, tag="xtp")
                    nc.sync.dma_start(
                        xq_t[:],
                        xtqb[cc * 128:(cc + 1) * 128, gp * 512:(gp + 1) * 512],
                    )
                    xqt.append(xq_t)
                xvt = []
                for cc in range(4):
                    xv_t = p1pool.tile([128, 512], F32R, tag="xvp")
                    nc.sync.dma_start(
                        xv_t[:],
                        xtq[cc * 128:(cc + 1) * 128, gp * 512:(gp + 1) * 512].bitcast(F32R),
                    )
                    xvt.append(xv_t)
                for t2 in range(4):
                    t = gp * 4 + t2
                    ps_q = ps1.tile([128, D], F32, tag="psk")
                    for cc in range(4):
                        nc.tensor.matmul(
                            ps_q[:],
                            lhsT=xqt[cc][:, t2 * 128:(t2 + 1) * 128],
                            rhs=wq_s[:, cc, :],
                            start=(cc == 0),
                            stop=(cc == 3),
                        )
                    nc.vector.tensor_copy(q_s[:, t, :], ps_q[:])
                    ps_v = ps1.tile([128, D], F32, tag="psk")
                    for cc in range(4):
                        nc.tensor.matmul(
                            ps_v[:],
                            lhsT=xvt[cc][:, t2 * 128:(t2 + 1) * 128],
                            rhs=wv_s[:, cc, :],
                            start=(cc == 0),
                            stop=(cc == 3),
                        )
                    nc.vector.tensor_copy(v_sb[:, t, :], ps_v[:])

        if stage == "p1":
            with tc.tile_pool(name="dbg", bufs=2) as dbg:
                for t in range(16):
                    dt_ = dbg.tile([128, D], F32, tag="dbg")
                    nc.vector.tensor_copy(dt_[:], km_s[:, t, :])
                    nc.sync.dma_start(loutd[t * 128:(t + 1) * 128, :], dt_[:])
            bigctx.close()

        # ---- Phase 2: agg = adj_block @ k_m ; w = inv_scale * rowdot(q, agg)
        if stage != "p1":
            with (
                tc.tile_pool(name="s2", bufs=12) as s2pool,
                tc.tile_pool(name="scrp", bufs=4) as scrpool,
                tc.tile_pool(name="ps2", bufs=8, space="PSUM") as ps2,
            ):
                for qq in range(4):  # quarters of 512 own rows -> 4 PSUM banks
                    aggs = [ps2.tile([128, D], F32, tag="agg", name=f"agg_{qq}_{i}") for i in range(4)]
                    for jb in range(16):  # batches of 4 j-chunks (512 KB DMA)
                        at = s2pool.tile([128, 4, 512], FP8, tag="adjs")
                        nc.sync.dma_start(
                            at[:],
                            adjt[jb * 512:(jb + 1) * 512, qq * 512:(qq + 1) * 512]
                            .rearrange("(jc2 p) i -> p jc2 i", p=128),
                        )
                        for u in range(2):  # chunk pairs -> fp8 DoubleRow (K=256/MM)
                            jp2 = jb * 2 + u
                            for is_ in range(4):
                                nc.tensor.matmul(
                                    aggs[is_][:],
                                    lhsT=at[:, 2 * u:2 * u + 2, is_ * 128:(is_ + 1) * 128],
                                    rhs=km_s[:, 4 * jb + 2 * u:4 * jb + 2 * u + 2, :],
                                    start=(jp2 == 0),
                                    stop=(jp2 == 31),
                                    perf_mode=mybir.MatmulPerfMode.DoubleRow,
                                )
                    for is_ in range(4):
                        t = qq * 4 + is_
                        # NOTE: tensor_tensor_reduce with a PSUM in0 faults the
                        # device (HW-only, sim-clean) — use mul + reduce instead.
                        scr = scrpool.tile([128, D], F32, tag="scr")
                        nc.vector.tensor_mul(scr[:], aggs[is_][:], q_s[:, t, :])
                        nc.vector.reduce_sum(out=w_sb[:, t:t + 1], in_=scr[:], axis=X_AX)

            bigctx.close()  # frees km/q/wq/wk (128+32+16 KB/partition) for the tail

            # ---- Phase 3: mask own w (and inv_scale), publish, AllGather --
            nc.vector.tensor_scalar_mul(w2[:], w_sb[:], INV_SCALE)
            nc.vector.tensor_mul(w2[:], w2[:], maskown_s[:])
            if stage == "p2":
                nc.sync.dma_start(loutd[0:128, 0:16], w2[:])
            else:
                nc.sync.dma_start(w_locd[:], w2[:])
                nc.gpsimd.collective_compute(
                    "AllGather",
                    mybir.AluOpType.bypass,
                    replica_groups=[[0, 1, 2, 3, 4, 5, 6, 7]],
                    ins=[w_locd[:]],
                    outs=[w_alld[:]],
                )

        if stage == "coll":
            with tc.tile_pool(name="dbg2", bufs=1) as dbg2:
                wdbg = dbg2.tile([128, 128], F32)
                nc.sync.dma_start(
                    wdbg[:].rearrange("p (g t) -> p g t", g=8),
                    w_alld[:].rearrange("g p t -> p g t"),
                )
                nc.sync.dma_start(loutd[0:128, 0:128], wdbg[:])

        if stage == "full":
            with (
                tc.tile_pool(name="tail", bufs=1) as tailp,
                tc.tile_pool(name="ltp", bufs=3) as ltp,
            ):
                # ---- Phase 4: softmax normalizers for both batches --------
                wall = tailp.tile([128, 128], F32)
                nc.sync.dma_start(
                    wall[:].rearrange("p (g t) -> p g t", g=8),
                    w_alld[:].rearrange("g p t -> p g t"),
                )
                gnegs, rinvs = [], []
                for h in range(2):
                    wh = wall[:, h * 64:(h + 1) * 64]
                    m1 = tailp.tile([128, 1], F32, tag=f"m1_{h}", name=f"m1_{h}")
                    nc.vector.reduce_max(out=m1[:], in_=wh, axis=X_AX)
                    gmax = tailp.tile([128, 1], F32, tag=f"gmax_{h}", name=f"gmax_{h}")
                    nc.gpsimd.partition_all_reduce(
                        gmax[:], m1[:], channels=128, reduce_op=bass_isa.ReduceOp.max
                    )
                    gneg = tailp.tile([128, 1], F32, tag=f"gneg_{h}", name=f"gneg_{h}")
                    nc.vector.tensor_scalar_mul(gneg[:], gmax[:], -1.0)
                    eh = tailp.tile([128, 64], F32, tag=f"eh_{h}", name=f"eh_{h}")
                    nc.scalar.activation(
                        out=eh[:], in_=wh, func=mybir.ActivationFunctionType.Exp,
                        bias=gneg[:], scale=1.0,
                    )
                    s1 = tailp.tile([128, 1], F32, tag=f"s1_{h}", name=f"s1_{h}")
                    nc.vector.reduce_sum(out=s1[:], in_=eh[:], axis=X_AX)
                    gsum = tailp.tile([128, 1], F32, tag=f"gsum_{h}", name=f"gsum_{h}")
                    nc.gpsimd.partition_all_reduce(
                        gsum[:], s1[:], channels=128, reduce_op=bass_isa.ReduceOp.add
                    )
                    rinv = tailp.tile([128, 1], F32, tag=f"rinv_{h}", name=f"rinv_{h}")
                    nc.vector.reciprocal(rinv[:], gsum[:])
                    gnegs.append(gneg)
                    rinvs.append(rinv)

                # select my batch's normalizers via bsel (per-core input)
                ga = tailp.tile([128, 1], F32, tag="ga")
                gb = tailp.tile([128, 1], F32, tag="gb")
                gneg_my = tailp.tile([128, 1], F32, tag="gneg_my")
                nc.vector.tensor_mul(ga[:], gnegs[0][:], bsel_s[:, 0:1])
                nc.vector.tensor_mul(gb[:], gnegs[1][:], bsel_s[:, 1:2])
                nc.vector.tensor_add(gneg_my[:], ga[:], gb[:])
                ra = tailp.tile([128, 1], F32, tag="ra")
                rb = tailp.tile([128, 1], F32, tag="rb")
                rinv_my = tailp.tile([128, 1], F32, tag="rinv_my")
                nc.vector.tensor_mul(ra[:], rinvs[0][:], bsel_s[:, 0:1])
                nc.vector.tensor_mul(rb[:], rinvs[1][:], bsel_s[:, 1:2])
                nc.vector.tensor_add(rinv_my[:], ra[:], rb[:])

                eown = tailp.tile([128, 16], F32, tag="eown")
                nc.scalar.activation(
                    out=eown[:], in_=w2[:], func=mybir.ActivationFunctionType.Exp,
                    bias=gneg_my[:], scale=1.0,
                )
                pown = tailp.tile([128, 16], F32, tag="pown")
                nc.vector.tensor_scalar_mul(pown[:], eown[:], rinv_my[:])

                # ---- Phase 5b: L rows = p_i * v_i -------------------------
                for t in range(16):
                    lt = ltp.tile([128, D], F32, tag="lt")
                    nc.vector.tensor_scalar_mul(lt[:], v_sb[:, t, :], pown[:, t:t + 1])
                    nc.sync.dma_start(loutd[t * 128:(t + 1) * 128, :], lt[:])

    nc.finalize()
    return nc


def _prep_inputs(X, adj, mask, Wqk, Wv):
    import ml_dtypes
    bf16 = ml_dtypes.bfloat16
    fp8 = ml_dtypes.float8_e4m3
    X = np.ascontiguousarray(np.asarray(X, dtype=np.float32))
    adj = np.asarray(adj, dtype=np.float32)
    mask = np.ascontiguousarray(np.asarray(mask, dtype=np.float32))
    Wqk = np.asarray(Wqk, dtype=np.float32)
    Wv = np.ascontiguousarray(np.asarray(Wv, dtype=np.float32))
    wq_h = np.ascontiguousarray(Wqk[:, :D].astype(bf16))
    wk_h = np.ascontiguousarray(Wqk[:, D:].astype(bf16))

    in_maps = []
    for b in range(B):
        xt_b = np.ascontiguousarray(X[b].T)
        xt_bh = np.ascontiguousarray(xt_b.astype(bf16))
        adjt_bh = np.ascontiguousarray(adj[b].astype(fp8).T)
        maskq_b = np.ascontiguousarray(mask[b].reshape(64, 128).T)
        for r in range(4):
            i0 = r * RPC
            bsel = np.zeros((128, 2), np.float32)
            bsel[:, b] = 1.0
            in_maps.append({
                "adjt": np.ascontiguousarray(adjt_bh[:, i0:i0 + RPC]),
                "xt": xt_bh,
                "xtq": np.ascontiguousarray(xt_b[:, i0:i0 + RPC]),
                "xtqb": np.ascontiguousarray(xt_bh[:, i0:i0 + RPC]),
                "wq": wq_h,
                "wk": wk_h,
                "wv": Wv,
                "maskq": maskq_b,
                "maskown": np.ascontiguousarray(mask[b, i0:i0 + RPC].reshape(16, 128).T),
                "bsel": bsel,
            })
    return in_maps


def _run(inputs, **kwargs):
    if "nc" not in _CACHE:
        _CACHE["nc"] = _build()
    nc = _CACHE["nc"]
    in_maps = _prep_inputs(**inputs)
    res = run_bass_kernel_spmd(nc, in_maps, list(range(8)), **kwargs)
    L = np.empty((B, N, D), np.float32)
    for c in range(8):
        b, r = divmod(c, 4)
        L[b, r * RPC:(r + 1) * RPC] = res.results[c]["lout"]
    return L, res


def kernel(X, adj, mask, Wqk, Wv):
    L, _ = _run(dict(X=X, adj=adj, mask=mask, Wqk=Wqk, Wv=Wv))
    return L


# revision 24
# speedup vs baseline: 1.4208x; 1.0324x over previous
"""CAMIL self-attention Trainium2 kernel (8 NeuronCores, SPMD).

Reference computation (B=2, N=8192, IN_DIM=ATT_DIM=512):
    q = X @ Wq ; k = X @ Wk ; v = X @ Wv
    w_i = inv_scale * m_i * sum_d q[i,d] * (adj @ (k*m))[i,d]
    L   = softmax(w, axis=bag)[:, :, None] * v

Sharding: 8 cores = (batch 2) x (4 row-blocks of 2048). Each core holds
adj[b, i_block, :]^T (pre-transposed on host so the contraction dim j lands
on SBUF partitions), computes its w-slice on-device, the 8 cores AllGather
w (2 KB/core), every core computes the softmax normalizers for both batches
locally, then scales its own v rows.

The big matmul (adj^T-tile stationary, k_m moving, fp32r) runs at 1 cyc/row,
so the kernel sits near the HBM roofline for streaming the 512 MB adj.
v is computed with plain fp32 matmuls (it multiplies the near-one-hot softmax
directly, so it carries the output's absmax precision).
"""

import numpy as np
from contextlib import ExitStack

import concourse.bass as bass
import concourse.bacc as bacc
import concourse.tile as tile
from concourse import mybir, bass_isa
from concourse.bass_utils import run_bass_kernel_spmd

F32 = mybir.dt.float32
F32R = mybir.dt.float32r
BF16 = mybir.dt.bfloat16
FP8 = mybir.dt.float8e4

B, N, D = 2, 8192, 512
RPC = N // 4  # rows per core: 2048
INV_SCALE = float(1.0 / np.sqrt(np.float32(D)))

_CACHE = {}


def _build(stage="full"):
    # stage: debug gate — "p1" (projections only), "p2" (+w), "coll" (+AllGather),
    # "full" (everything). kernel() always uses "full".
    nc = bacc.Bacc(None, target_bir_lowering=False, debug=False, num_devices=8)

    adjt = nc.dram_tensor("adjt", [N, RPC], FP8, kind="ExternalInput")
    xt = nc.dram_tensor("xt", [D, N], BF16, kind="ExternalInput")
    xtq = nc.dram_tensor("xtq", [D, RPC], F32, kind="ExternalInput")
    xtqb = nc.dram_tensor("xtqb", [D, RPC], BF16, kind="ExternalInput")
    wqd = nc.dram_tensor("wq", [D, D], BF16, kind="ExternalInput")
    wkd = nc.dram_tensor("wk", [D, D], BF16, kind="ExternalInput")
    wvd = nc.dram_tensor("wv", [D, D], F32, kind="ExternalInput")
    maskqd = nc.dram_tensor("maskq", [128, 64], F32, kind="ExternalInput")
    maskownd = nc.dram_tensor("maskown", [128, 16], F32, kind="ExternalInput")
    bseld = nc.dram_tensor("bsel", [128, 2], F32, kind="ExternalInput")
    loutd = nc.dram_tensor("lout", [RPC, D], F32, kind="ExternalOutput")

    w_locd = nc.dram_tensor("w_loc", [128, 16], F32)
    w_alld = nc.dram_tensor("w_all", [8, 128, 16], F32, addr_space="Shared")

    MUL = mybir.AluOpType.mult
    ADD = mybir.AluOpType.add
    X_AX = mybir.AxisListType.X

    with tile.TileContext(nc) as tc, ExitStack() as ctx:
        wtail = ctx.enter_context(tc.tile_pool(name="wtail", bufs=1))
        bigctx = ExitStack()
        big = bigctx.enter_context(tc.tile_pool(name="big", bufs=1))

        km_s = big.tile([128, 64, D], FP8)       # k*mask, [j-part, j-chunk, d]
        q_s = big.tile([128, 16, D], F32)        # own q rows
        wq_s = big.tile([128, 4, D], BF16)
        wk_s = big.tile([128, 4, D], BF16)
        maskq_s = big.tile([128, 64], F32)
        wv_s = big.tile([128, 4, D], F32R)

        w_sb = wtail.tile([128, 16], F32)        # own w (pre-mask)
        w2 = wtail.tile([128, 16], F32)          # own w (masked)
        maskown_s = wtail.tile([128, 16], F32)
        v_sb = wtail.tile([128, 16, D], F32)     # own v rows (fp32r matmul)
        bsel_s = wtail.tile([128, 2], F32)

        nc.sync.dma_start(wq_s[:], wqd[:].rearrange("(cc p) d -> p cc d", p=128))
        nc.sync.dma_start(wk_s[:], wkd[:].rearrange("(cc p) d -> p cc d", p=128))
        nc.sync.dma_start(maskq_s[:], maskqd[:])
        nc.sync.dma_start(wv_s[:], wvd[:].bitcast(F32R).rearrange("(cc p) d -> p cc d", p=128))
        nc.sync.dma_start(maskown_s[:], maskownd[:])
        nc.sync.dma_start(bsel_s[:], bseld[:])

        # ---- Phase 1: k_m (all N rows) and q (own rows) projections -------
        with (
            tc.tile_pool(name="p1", bufs=16) as p1pool,
            tc.tile_pool(name="ps1", bufs=6, space="PSUM") as ps1,
        ):
            for jp in range(16):  # panels of 512 bag rows
                xtiles = []
                for cc in range(4):
                    xt_t = p1pool.tile([128, 512], BF16, tag="xtp")
                    nc.sync.dma_start(
                        xt_t[:],
                        xt[cc * 128:(cc + 1) * 128, jp * 512:(jp + 1) * 512],
                    )
                    xtiles.append(xt_t)
                for jc2 in range(4):
                    jc = jp * 4 + jc2
                    ps_k = ps1.tile([128, D], F32, tag="psk")
                    for cc in range(4):
                        nc.tensor.matmul(
                            ps_k[:],
                            lhsT=xtiles[cc][:, jc2 * 128:(jc2 + 1) * 128],
                            rhs=wk_s[:, cc, :],
                            start=(cc == 0),
                            stop=(cc == 3),
                        )
                    nc.vector.tensor_scalar_mul(km_s[:, jc, :], ps_k[:], maskq_s[:, jc:jc + 1])
            for gp in range(4):  # own panels of 512 rows
                xqt = []
                for cc in range(4):
                    xq_t = p1pool.tile([128, 512], BF16, tag="xtp")
                    nc.sync.dma_start(
                        xq_t[:],
                        xtqb[cc * 128:(cc + 1) * 128, gp * 512:(gp + 1) * 512],
                    )
                    xqt.append(xq_t)
                xvt = []
                for cc in range(4):
                    xv_t = p1pool.tile([128, 512], F32R, tag="xvp")
                    nc.sync.dma_start(
                        xv_t[:],
                        xtq[cc * 128:(cc + 1) * 128, gp * 512:(gp + 1) * 512].bitcast(F32R),
                    )
                    xvt.append(xv_t)
                for t2 in range(4):
                    t = gp * 4 + t2
                    ps_q = ps1.tile([128, D], F32, tag="psk")
                    for cc in range(4):
                        nc.tensor.matmul(
                            ps_q[:],
                            lhsT=xqt[cc][:, t2 * 128:(t2 + 1) * 128],
                            rhs=wq_s[:, cc, :],
                            start=(cc == 0),
                            stop=(cc == 3),
                        )
                    nc.vector.tensor_copy(q_s[:, t, :], ps_q[:])
                for t2 in range(4):
                    t = gp * 4 + t2
                    ps_v = ps1.tile([128, D], F32, tag="psk")
                    for cc in range(4):
                        nc.tensor.matmul(
                            ps_v[:],
                            lhsT=xvt[cc][:, t2 * 128:(t2 + 1) * 128],
                            rhs=wv_s[:, cc, :],
                            start=(cc == 0),
                            stop=(cc == 3),
                        )
                    nc.vector.tensor_copy(v_sb[:, t, :], ps_v[:])

        if stage == "p1":
            with tc.tile_pool(name="dbg", bufs=2) as dbg:
                for t in range(16):
                    dt_ = dbg.tile([128, D], F32, tag="dbg")
                    nc.vector.tensor_copy(dt_[:], km_s[:, t, :])
                    nc.sync.dma_start(loutd[t * 128:(t + 1) * 128, :], dt_[:])
            bigctx.close()

        # ---- Phase 2: agg = adj_block @ k_m ; w = inv_scale * rowdot(q, agg)
        if stage != "p1":
            with (
                tc.tile_pool(name="s2", bufs=16) as s2pool,
                tc.tile_pool(name="scrp", bufs=4) as scrpool,
                tc.tile_pool(name="ps2", bufs=8, space="PSUM") as ps2,
            ):
                for qq in range(4):  # quarters of 512 own rows -> 4 PSUM banks
                    aggs = [ps2.tile([128, D], F32, tag="agg", name=f"agg_{qq}_{i}") for i in range(4)]
                    for jb in range(16):  # batches of 4 j-chunks (512 KB DMA)
                        at = s2pool.tile([128, 4, 512], FP8, tag="adjs")
                        nc.sync.dma_start(
                            at[:],
                            adjt[jb * 512:(jb + 1) * 512, qq * 512:(qq + 1) * 512]
                            .rearrange("(jc2 p) i -> p jc2 i", p=128),
                        )
                        for u in range(2):  # chunk pairs -> fp8 DoubleRow (K=256/MM)
                            jp2 = jb * 2 + u
                            for is_ in range(4):
                                nc.tensor.matmul(
                                    aggs[is_][:],
                                    lhsT=at[:, 2 * u:2 * u + 2, is_ * 128:(is_ + 1) * 128],
                                    rhs=km_s[:, 4 * jb + 2 * u:4 * jb + 2 * u + 2, :],
                                    start=(jp2 == 0),
                                    stop=(jp2 == 31),
                                    perf_mode=mybir.MatmulPerfMode.DoubleRow,
                                )
                    for is_ in range(4):
                        t = qq * 4 + is_
                        # NOTE: tensor_tensor_reduce with a PSUM in0 faults the
                        # device (HW-only, sim-clean) — use mul + reduce instead.
                        scr = scrpool.tile([128, D], F32, tag="scr")
                        nc.vector.tensor_mul(scr[:], aggs[is_][:], q_s[:, t, :])
                        nc.vector.reduce_sum(out=w_sb[:, t:t + 1], in_=scr[:], axis=X_AX)

            bigctx.close()  # frees km/q/wq/wk (128+32+16 KB/partition) for the tail

            # ---- Phase 3: mask own w (and inv_scale), publish, AllGather --
            nc.vector.tensor_scalar_mul(w2[:], w_sb[:], INV_SCALE)
            nc.vector.tensor_mul(w2[:], w2[:], maskown_s[:])
            if stage == "p2":
                nc.sync.dma_start(loutd[0:128, 0:16], w2[:])
            else:
                nc.sync.dma_start(w_locd[:], w2[:])
                nc.gpsimd.collective_compute(
                    "AllGather",
                    mybir.AluOpType.bypass,
                    replica_groups=[[0, 1, 2, 3, 4, 5, 6, 7]],
                    ins=[w_locd[:]],
                    outs=[w_alld[:]],
                )

        if stage == "coll":
            with tc.tile_pool(name="dbg2", bufs=1) as dbg2:
                wdbg = dbg2.tile([128, 128], F32)
                nc.sync.dma_start(
                    wdbg[:].rearrange("p (g t) -> p g t", g=8),
                    w_alld[:].rearrange("g p t -> p g t"),
                )
                nc.sync.dma_start(loutd[0:128, 0:128], wdbg[:])

        if stage == "full":
            with (
                tc.tile_pool(name="tail", bufs=1) as tailp,
                tc.tile_pool(name="ltp", bufs=3) as ltp,
            ):
                # ---- Phase 4: softmax normalizers for both batches --------
                wall = tailp.tile([128, 128], F32)
                nc.sync.dma_start(
                    wall[:].rearrange("p (g t) -> p g t", g=8),
                    w_alld[:].rearrange("g p t -> p g t"),
                )
                gnegs, rinvs = [], []
                for h in range(2):
                    wh = wall[:, h * 64:(h + 1) * 64]
                    m1 = tailp.tile([128, 1], F32, tag=f"m1_{h}", name=f"m1_{h}")
                    nc.vector.reduce_max(out=m1[:], in_=wh, axis=X_AX)
                    gmax = tailp.tile([128, 1], F32, tag=f"gmax_{h}", name=f"gmax_{h}")
                    nc.gpsimd.partition_all_reduce(
                        gmax[:], m1[:], channels=128, reduce_op=bass_isa.ReduceOp.max
                    )
                    gneg = tailp.tile([128, 1], F32, tag=f"gneg_{h}", name=f"gneg_{h}")
                    nc.vector.tensor_scalar_mul(gneg[:], gmax[:], -1.0)
                    eh = tailp.tile([128, 64], F32, tag=f"eh_{h}", name=f"eh_{h}")
                    nc.scalar.activation(
                        out=eh[:], in_=wh, func=mybir.ActivationFunctionType.Exp,
                        bias=gneg[:], scale=1.0,
                    )
                    s1 = tailp.tile([128, 1], F32, tag=f"s1_{h}", name=f"s1_{h}")
                    nc.vector.reduce_sum(out=s1[:], in_=eh[:], axis=X_AX)
                    gsum = tailp.tile([128, 1], F32, tag=f"gsum_{h}", name=f"gsum_{h}")
                    nc.gpsimd.partition_all_reduce(
                        gsum[:], s1[:], channels=128, reduce_op=bass_isa.ReduceOp.add
                    )
                    rinv = tailp.tile([128, 1], F32, tag=f"rinv_{h}", name=f"rinv_{h}")
                    nc.vector.reciprocal(rinv[:], gsum[:])
                    gnegs.append(gneg)
                    rinvs.append(rinv)

                # select my batch's normalizers via bsel (per-core input)
                ga = tailp.tile([128, 1], F32, tag="ga")
                gb = tailp.tile([128, 1], F32, tag="gb")
                gneg_my = tailp.tile([128, 1], F32, tag="gneg_my")
                nc.vector.tensor_mul(ga[:], gnegs[0][:], bsel_s[:, 0:1])
                nc.vector.tensor_mul(gb[:], gnegs[1][:], bsel_s[:, 1:2])
                nc.vector.tensor_add(gneg_my[:], ga[:], gb[:])
                ra = tailp.tile([128, 1], F32, tag="ra")
                rb = tailp.tile([128, 1], F32, tag="rb")
                rinv_my = tailp.tile([128, 1], F32, tag="rinv_my")
                nc.vector.tensor_mul(ra[:], rinvs[0][:], bsel_s[:, 0:1])
                nc.vector.tensor_mul(rb[:], rinvs[1][:], bsel_s[:, 1:2])
                nc.vector.tensor_add(rinv_my[:], ra[:], rb[:])

                eown = tailp.tile([128, 16], F32, tag="eown")
                nc.scalar.activation(
                    out=eown[:], in_=w2[:], func=mybir.ActivationFunctionType.Exp,
                    bias=gneg_my[:], scale=1.0,
                )
                pown = tailp.tile([128, 16], F32, tag="pown")
                nc.vector.tensor_scalar_mul(pown[:], eown[:], rinv_my[:])

                # ---- Phase 5b: L rows = p_i * v_i -------------------------
                for t in range(16):
                    lt = ltp.tile([128, D], F32, tag="lt")
                    nc.vector.tensor_scalar_mul(lt[:], v_sb[:, t, :], pown[:, t:t + 1])
                    nc.sync.dma_start(loutd[t * 128:(t + 1) * 128, :], lt[:])

    nc.finalize()
    return nc


def _prep_inputs(X, adj, mask, Wqk, Wv):
    import ml_dtypes
    bf16 = ml_dtypes.bfloat16
    fp8 = ml_dtypes.float8_e4m3
    X = np.ascontiguousarray(np.asarray(X, dtype=np.float32))
    adj = np.asarray(adj, dtype=np.float32)
    mask = np.ascontiguousarray(np.asarray(mask, dtype=np.float32))
    Wqk = np.asarray(Wqk, dtype=np.float32)
    Wv = np.ascontiguousarray(np.asarray(Wv, dtype=np.float32))
    wq_h = np.ascontiguousarray(Wqk[:, :D].astype(bf16))
    wk_h = np.ascontiguousarray(Wqk[:, D:].astype(bf16))

    in_maps = []
    for b in range(B):
        xt_b = np.ascontiguousarray(X[b].T)
        xt_bh = np.ascontiguousarray(xt_b.astype(bf16))
        adjt_bh = np.ascontiguousarray(adj[b].astype(fp8).T)
        maskq_b = np.ascontiguousarray(mask[b].reshape(64, 128).T)
        for r in range(4):
            i0 = r * RPC
            bsel = np.zeros((128, 2), np.float32)
            bsel[:, b] = 1.0
            in_maps.append({
                "adjt": np.ascontiguousarray(adjt_bh[:, i0:i0 + RPC]),
                "xt": xt_bh,
                "xtq": np.ascontiguousarray(xt_b[:, i0:i0 + RPC]),
                "xtqb": np.ascontiguousarray(xt_bh[:, i0:i0 + RPC]),
                "wq": wq_h,
                "wk": wk_h,
                "wv": Wv,
                "maskq": maskq_b,
                "maskown": np.ascontiguousarray(mask[b, i0:i0 + RPC].reshape(16, 128).T),
                "bsel": bsel,
            })
    return in_maps


def _run(inputs, **kwargs):
    if "nc" not in _CACHE:
        _CACHE["nc"] = _build()
    nc = _CACHE["nc"]
    in_maps = _prep_inputs(**inputs)
    res = run_bass_kernel_spmd(nc, in_maps, list(range(8)), **kwargs)
    L = np.empty((B, N, D), np.float32)
    for c in range(8):
        b, r = divmod(c, 4)
        L[b, r * RPC:(r + 1) * RPC] = res.results[c]["lout"]
    return L, res


def kernel(X, adj, mask, Wqk, Wv):
    L, _ = _run(dict(X=X, adj=adj, mask=mask, Wqk=Wqk, Wv=Wv))
    return L


# revision 26
# speedup vs baseline: 1.4799x; 1.0416x over previous
"""CAMIL self-attention Trainium2 kernel (8 NeuronCores, SPMD).

Reference computation (B=2, N=8192, IN_DIM=ATT_DIM=512):
    q = X @ Wq ; k = X @ Wk ; v = X @ Wv
    w_i = inv_scale * m_i * sum_d q[i,d] * (adj @ (k*m))[i,d]
    L   = softmax(w, axis=bag)[:, :, None] * v

Sharding: 8 cores = (batch 2) x (4 row-blocks of 2048). Each core holds
adj[b, i_block, :]^T (pre-transposed on host so the contraction dim j lands
on SBUF partitions), computes its w-slice on-device, the 8 cores AllGather
w (2 KB/core), every core computes the softmax normalizers for both batches
locally, then scales its own v rows.

The big matmul (adj^T-tile stationary, k_m moving, fp32r) runs at 1 cyc/row,
so the kernel sits near the HBM roofline for streaming the 512 MB adj.
v is computed with plain fp32 matmuls (it multiplies the near-one-hot softmax
directly, so it carries the output's absmax precision).
"""

import numpy as np
from contextlib import ExitStack

import concourse.bass as bass
import concourse.bacc as bacc
import concourse.tile as tile
from concourse import mybir, bass_isa
from concourse.bass_utils import run_bass_kernel_spmd

F32 = mybir.dt.float32
F32R = mybir.dt.float32r
BF16 = mybir.dt.bfloat16
FP8 = mybir.dt.float8e4

B, N, D = 2, 8192, 512
RPC = N // 4  # rows per core: 2048
INV_SCALE = float(1.0 / np.sqrt(np.float32(D)))

_CACHE = {}


def _build(stage="full"):
    # stage: debug gate — "p1" (projections only), "p2" (+w), "coll" (+AllGather),
    # "full" (everything). kernel() always uses "full".
    nc = bacc.Bacc(None, target_bir_lowering=False, debug=False, num_devices=8)

    adjt = nc.dram_tensor("adjt", [N, RPC], FP8, kind="ExternalInput")
    xt = nc.dram_tensor("xt", [D, N], BF16, kind="ExternalInput")
    xtq = nc.dram_tensor("xtq", [D, RPC], F32, kind="ExternalInput")
    xtqb = nc.dram_tensor("xtqb", [D, RPC], BF16, kind="ExternalInput")
    wqd = nc.dram_tensor("wq", [D, D], BF16, kind="ExternalInput")
    wkd = nc.dram_tensor("wk", [D, D], BF16, kind="ExternalInput")
    wvd = nc.dram_tensor("wv", [D, D], F32, kind="ExternalInput")
    maskqd = nc.dram_tensor("maskq", [128, 64], F32, kind="ExternalInput")
    maskownd = nc.dram_tensor("maskown", [128, 16], F32, kind="ExternalInput")
    bseld = nc.dram_tensor("bsel", [128, 2], F32, kind="ExternalInput")
    loutd = nc.dram_tensor("lout", [RPC, D], F32, kind="ExternalOutput")

    w_locd = nc.dram_tensor("w_loc", [128, 16], F32)
    w_alld = nc.dram_tensor("w_all", [8, 128, 16], F32, addr_space="Shared")
    dwarm_in = nc.dram_tensor("dwarm_in", [1, 16], F32)
    dwarm_out = nc.dram_tensor("dwarm_out", [8, 1, 16], F32, addr_space="Shared")

    MUL = mybir.AluOpType.mult
    ADD = mybir.AluOpType.add
    X_AX = mybir.AxisListType.X

    with tile.TileContext(nc) as tc, ExitStack() as ctx:
        wtail = ctx.enter_context(tc.tile_pool(name="wtail", bufs=1))
        bigctx = ExitStack()
        big = bigctx.enter_context(tc.tile_pool(name="big", bufs=1))

        km_s = big.tile([128, 64, D], FP8)       # k*mask, [j-part, j-chunk, d]
        q_s = big.tile([128, 16, D], F32)        # own q rows
        wq_s = big.tile([128, 4, D], BF16)
        wk_s = big.tile([128, 4, D], BF16)
        maskq_s = big.tile([128, 64], F32)
        wv_s = big.tile([128, 4, D], F32R)

        w_sb = wtail.tile([128, 16], F32)        # own w (pre-mask)
        w2 = wtail.tile([128, 16], F32)          # own w (masked)
        maskown_s = wtail.tile([128, 16], F32)
        v_sb = wtail.tile([128, 16, D], F32)     # own v rows (fp32r matmul)
        bsel_s = wtail.tile([128, 2], F32)

        nc.sync.dma_start(wq_s[:], wqd[:].rearrange("(cc p) d -> p cc d", p=128))
        nc.sync.dma_start(wk_s[:], wkd[:].rearrange("(cc p) d -> p cc d", p=128))
        nc.sync.dma_start(maskq_s[:], maskqd[:])
        nc.sync.dma_start(wv_s[:], wvd[:].bitcast(F32R).rearrange("(cc p) d -> p cc d", p=128))
        nc.sync.dma_start(maskown_s[:], maskownd[:])
        nc.sync.dma_start(bsel_s[:], bseld[:])

        # warmups, off the critical path: exp LUT load + collective firmware
        warm = wtail.tile([128, 16], F32)
        nc.vector.memset(warm[:], 0.0)
        nc.scalar.activation(out=warm[:], in_=warm[:],
                             func=mybir.ActivationFunctionType.Exp, bias=0.0, scale=1.0)
        nc.sync.dma_start(dwarm_in[:], warm[0:1, :])
        nc.gpsimd.collective_compute(
            "AllGather",
            mybir.AluOpType.bypass,
            replica_groups=[[0, 1, 2, 3, 4, 5, 6, 7]],
            ins=[dwarm_in[:]],
            outs=[dwarm_out[:]],
        )

        # ---- Phase 1: k_m (all N rows) and q (own rows) projections -------
        with (
            tc.tile_pool(name="p1", bufs=16) as p1pool,
            tc.tile_pool(name="ps1", bufs=6, space="PSUM") as ps1,
        ):
            for jp in range(16):  # panels of 512 bag rows
                xtiles = []
                for cc in range(4):
                    xt_t = p1pool.tile([128, 512], BF16, tag="xtp")
                    nc.sync.dma_start(
                        xt_t[:],
                        xt[cc * 128:(cc + 1) * 128, jp * 512:(jp + 1) * 512],
                    )
                    xtiles.append(xt_t)
                for jc2 in range(4):
                    jc = jp * 4 + jc2
                    ps_k = ps1.tile([128, D], F32, tag="psk")
                    for cc in range(4):
                        nc.tensor.matmul(
                            ps_k[:],
                            lhsT=xtiles[cc][:, jc2 * 128:(jc2 + 1) * 128],
                            rhs=wk_s[:, cc, :],
                            start=(cc == 0),
                            stop=(cc == 3),
                        )
                    nc.vector.tensor_scalar_mul(km_s[:, jc, :], ps_k[:], maskq_s[:, jc:jc + 1])
            for gp in range(4):  # own panels of 512 rows
                xqt = []
                for cc in range(4):
                    xq_t = p1pool.tile([128, 512], BF16, tag="xtp")
                    nc.sync.dma_start(
                        xq_t[:],
                        xtqb[cc * 128:(cc + 1) * 128, gp * 512:(gp + 1) * 512],
                    )
                    xqt.append(xq_t)
                xvt = []
                for cc in range(4):
                    xv_t = p1pool.tile([128, 512], F32R, tag="xvp")
                    nc.sync.dma_start(
                        xv_t[:],
                        xtq[cc * 128:(cc + 1) * 128, gp * 512:(gp + 1) * 512].bitcast(F32R),
                    )
                    xvt.append(xv_t)
                for t2 in range(4):
                    t = gp * 4 + t2
                    ps_q = ps1.tile([128, D], F32, tag="psk")
                    for cc in range(4):
                        nc.tensor.matmul(
                            ps_q[:],
                            lhsT=xqt[cc][:, t2 * 128:(t2 + 1) * 128],
                            rhs=wq_s[:, cc, :],
                            start=(cc == 0),
                            stop=(cc == 3),
                        )
                    nc.vector.tensor_copy(q_s[:, t, :], ps_q[:])
                for t2 in range(4):
                    t = gp * 4 + t2
                    ps_v = ps1.tile([128, D], F32, tag="psk")
                    for cc in range(4):
                        nc.tensor.matmul(
                            ps_v[:],
                            lhsT=xvt[cc][:, t2 * 128:(t2 + 1) * 128],
                            rhs=wv_s[:, cc, :],
                            start=(cc == 0),
                            stop=(cc == 3),
                        )
                    nc.vector.tensor_copy(v_sb[:, t, :], ps_v[:])

        if stage == "p1":
            with tc.tile_pool(name="dbg", bufs=2) as dbg:
                for t in range(16):
                    dt_ = dbg.tile([128, D], F32, tag="dbg")
                    nc.vector.tensor_copy(dt_[:], km_s[:, t, :])
                    nc.sync.dma_start(loutd[t * 128:(t + 1) * 128, :], dt_[:])
            bigctx.close()

        # ---- Phase 2: agg = adj_block @ k_m ; w = inv_scale * rowdot(q, agg)
        if stage != "p1":
            with (
                tc.tile_pool(name="s2", bufs=16) as s2pool,
                tc.tile_pool(name="scrp", bufs=4) as scrpool,
                tc.tile_pool(name="ps2", bufs=8, space="PSUM") as ps2,
            ):
                for qq in range(4):  # quarters of 512 own rows -> 4 PSUM banks
                    aggs = [ps2.tile([128, D], F32, tag="agg", name=f"agg_{qq}_{i}") for i in range(4)]
                    for jb in range(16):  # batches of 4 j-chunks (512 KB DMA)
                        at = s2pool.tile([128, 4, 512], FP8, tag="adjs")
                        nc.sync.dma_start(
                            at[:],
                            adjt[jb * 512:(jb + 1) * 512, qq * 512:(qq + 1) * 512]
                            .rearrange("(jc2 p) i -> p jc2 i", p=128),
                        )
                        for u in range(2):  # chunk pairs -> fp8 DoubleRow (K=256/MM)
                            jp2 = jb * 2 + u
                            for is_ in range(4):
                                nc.tensor.matmul(
                                    aggs[is_][:],
                                    lhsT=at[:, 2 * u:2 * u + 2, is_ * 128:(is_ + 1) * 128],
                                    rhs=km_s[:, 4 * jb + 2 * u:4 * jb + 2 * u + 2, :],
                                    start=(jp2 == 0),
                                    stop=(jp2 == 31),
                                    perf_mode=mybir.MatmulPerfMode.DoubleRow,
                                )
                    for is_ in range(4):
                        t = qq * 4 + is_
                        # NOTE: tensor_tensor_reduce with a PSUM in0 faults the
                        # device (HW-only, sim-clean) — use mul + reduce instead.
                        scr = scrpool.tile([128, D], F32, tag="scr")
                        nc.vector.tensor_mul(scr[:], aggs[is_][:], q_s[:, t, :])
                        nc.vector.reduce_sum(out=w_sb[:, t:t + 1], in_=scr[:], axis=X_AX)

            bigctx.close()  # frees km/q/wq/wk (128+32+16 KB/partition) for the tail

            # ---- Phase 3: mask own w (and inv_scale), publish, AllGather --
            nc.vector.tensor_scalar_mul(w2[:], w_sb[:], INV_SCALE)
            nc.vector.tensor_mul(w2[:], w2[:], maskown_s[:])
            if stage == "p2":
                nc.sync.dma_start(loutd[0:128, 0:16], w2[:])
            else:
                nc.sync.dma_start(w_locd[:], w2[:])
                nc.gpsimd.collective_compute(
                    "AllGather",
                    mybir.AluOpType.bypass,
                    replica_groups=[[0, 1, 2, 3, 4, 5, 6, 7]],
                    ins=[w_locd[:]],
                    outs=[w_alld[:]],
                )

        if stage == "coll":
            with tc.tile_pool(name="dbg2", bufs=1) as dbg2:
                wdbg = dbg2.tile([128, 128], F32)
                nc.sync.dma_start(
                    wdbg[:].rearrange("p (g t) -> p g t", g=8),
                    w_alld[:].rearrange("g p t -> p g t"),
                )
                nc.sync.dma_start(loutd[0:128, 0:128], wdbg[:])

        if stage == "full":
            with (
                tc.tile_pool(name="tail", bufs=1) as tailp,
                tc.tile_pool(name="ltp", bufs=8) as ltp,
            ):
                # ---- Phase 4: softmax normalizers for both batches --------
                wall = tailp.tile([128, 128], F32)
                nc.sync.dma_start(
                    wall[:].rearrange("p (g t) -> p g t", g=8),
                    w_alld[:].rearrange("g p t -> p g t"),
                )
                gnegs, rinvs = [], []
                for h in range(2):
                    wh = wall[:, h * 64:(h + 1) * 64]
                    m1 = tailp.tile([128, 1], F32, tag=f"m1_{h}", name=f"m1_{h}")
                    nc.vector.reduce_max(out=m1[:], in_=wh, axis=X_AX)
                    gmax = tailp.tile([128, 1], F32, tag=f"gmax_{h}", name=f"gmax_{h}")
                    nc.gpsimd.partition_all_reduce(
                        gmax[:], m1[:], channels=128, reduce_op=bass_isa.ReduceOp.max
                    )
                    gneg = tailp.tile([128, 1], F32, tag=f"gneg_{h}", name=f"gneg_{h}")
                    nc.vector.tensor_scalar_mul(gneg[:], gmax[:], -1.0)
                    eh = tailp.tile([128, 64], F32, tag=f"eh_{h}", name=f"eh_{h}")
                    nc.scalar.activation(
                        out=eh[:], in_=wh, func=mybir.ActivationFunctionType.Exp,
                        bias=gneg[:], scale=1.0,
                    )
                    s1 = tailp.tile([128, 1], F32, tag=f"s1_{h}", name=f"s1_{h}")
                    nc.vector.reduce_sum(out=s1[:], in_=eh[:], axis=X_AX)
                    gsum = tailp.tile([128, 1], F32, tag=f"gsum_{h}", name=f"gsum_{h}")
                    nc.gpsimd.partition_all_reduce(
                        gsum[:], s1[:], channels=128, reduce_op=bass_isa.ReduceOp.add
                    )
                    rinv = tailp.tile([128, 1], F32, tag=f"rinv_{h}", name=f"rinv_{h}")
                    nc.vector.reciprocal(rinv[:], gsum[:])
                    gnegs.append(gneg)
                    rinvs.append(rinv)

                # select my batch's normalizers via bsel (per-core input)
                ga = tailp.tile([128, 1], F32, tag="ga")
                gb = tailp.tile([128, 1], F32, tag="gb")
                gneg_my = tailp.tile([128, 1], F32, tag="gneg_my")
                nc.vector.tensor_mul(ga[:], gnegs[0][:], bsel_s[:, 0:1])
                nc.vector.tensor_mul(gb[:], gnegs[1][:], bsel_s[:, 1:2])
                nc.vector.tensor_add(gneg_my[:], ga[:], gb[:])
                ra = tailp.tile([128, 1], F32, tag="ra")
                rb = tailp.tile([128, 1], F32, tag="rb")
                rinv_my = tailp.tile([128, 1], F32, tag="rinv_my")
                nc.vector.tensor_mul(ra[:], rinvs[0][:], bsel_s[:, 0:1])
                nc.vector.tensor_mul(rb[:], rinvs[1][:], bsel_s[:, 1:2])
                nc.vector.tensor_add(rinv_my[:], ra[:], rb[:])

                eown = tailp.tile([128, 16], F32, tag="eown")
                nc.scalar.activation(
                    out=eown[:], in_=w2[:], func=mybir.ActivationFunctionType.Exp,
                    bias=gneg_my[:], scale=1.0,
                )
                pown = tailp.tile([128, 16], F32, tag="pown")
                nc.vector.tensor_scalar_mul(pown[:], eown[:], rinv_my[:])

                # ---- Phase 5b: L rows = p_i * v_i -------------------------
                for t in range(16):
                    lt = ltp.tile([128, D], F32, tag="lt")
                    nc.vector.tensor_scalar_mul(lt[:], v_sb[:, t, :], pown[:, t:t + 1])
                    eng = nc.sync if t % 2 == 0 else nc.gpsimd
                    eng.dma_start(loutd[t * 128:(t + 1) * 128, :], lt[:])

    nc.finalize()
    return nc


def _prep_inputs(X, adj, mask, Wqk, Wv):
    import ml_dtypes
    bf16 = ml_dtypes.bfloat16
    fp8 = ml_dtypes.float8_e4m3
    X = np.ascontiguousarray(np.asarray(X, dtype=np.float32))
    adj = np.asarray(adj, dtype=np.float32)
    mask = np.ascontiguousarray(np.asarray(mask, dtype=np.float32))
    Wqk = np.asarray(Wqk, dtype=np.float32)
    Wv = np.ascontiguousarray(np.asarray(Wv, dtype=np.float32))
    wq_h = np.ascontiguousarray(Wqk[:, :D].astype(bf16))
    wk_h = np.ascontiguousarray(Wqk[:, D:].astype(bf16))

    in_maps = []
    for b in range(B):
        xt_b = np.ascontiguousarray(X[b].T)
        xt_bh = np.ascontiguousarray(xt_b.astype(bf16))
        adjt_bh = np.ascontiguousarray(adj[b].astype(fp8).T)
        maskq_b = np.ascontiguousarray(mask[b].reshape(64, 128).T)
        for r in range(4):
            i0 = r * RPC
            bsel = np.zeros((128, 2), np.float32)
            bsel[:, b] = 1.0
            in_maps.append({
                "adjt": np.ascontiguousarray(adjt_bh[:, i0:i0 + RPC]),
                "xt": xt_bh,
                "xtq": np.ascontiguousarray(xt_b[:, i0:i0 + RPC]),
                "xtqb": np.ascontiguousarray(xt_bh[:, i0:i0 + RPC]),
                "wq": wq_h,
                "wk": wk_h,
                "wv": Wv,
                "maskq": maskq_b,
                "maskown": np.ascontiguousarray(mask[b, i0:i0 + RPC].reshape(16, 128).T),
                "bsel": bsel,
            })
    return in_maps


def _run(inputs, **kwargs):
    if "nc" not in _CACHE:
        _CACHE["nc"] = _build()
    nc = _CACHE["nc"]
    in_maps = _prep_inputs(**inputs)
    res = run_bass_kernel_spmd(nc, in_maps, list(range(8)), **kwargs)
    L = np.empty((B, N, D), np.float32)
    for c in range(8):
        b, r = divmod(c, 4)
        L[b, r * RPC:(r + 1) * RPC] = res.results[c]["lout"]
    return L, res


def kernel(X, adj, mask, Wqk, Wv):
    L, _ = _run(dict(X=X, adj=adj, mask=mask, Wqk=Wqk, Wv=Wv))
    return L


# revision 27
# speedup vs baseline: 1.5159x; 1.0243x over previous
"""CAMIL self-attention Trainium2 kernel (8 NeuronCores, SPMD).

Reference computation (B=2, N=8192, IN_DIM=ATT_DIM=512):
    q = X @ Wq ; k = X @ Wk ; v = X @ Wv
    w_i = inv_scale * m_i * sum_d q[i,d] * (adj @ (k*m))[i,d]
    L   = softmax(w, axis=bag)[:, :, None] * v

Sharding: 8 cores = (batch 2) x (4 row-blocks of 2048). Each core holds
adj[b, i_block, :]^T (pre-transposed on host so the contraction dim j lands
on SBUF partitions), computes its w-slice on-device, the 8 cores AllGather
w (2 KB/core), every core computes the softmax normalizers for both batches
locally, then scales its own v rows.

The big matmul (adj^T-tile stationary, k_m moving, fp32r) runs at 1 cyc/row,
so the kernel sits near the HBM roofline for streaming the 512 MB adj.
v is computed with plain fp32 matmuls (it multiplies the near-one-hot softmax
directly, so it carries the output's absmax precision).
"""

import numpy as np
from contextlib import ExitStack

import concourse.bass as bass
import concourse.bacc as bacc
import concourse.tile as tile
from concourse import mybir, bass_isa
from concourse.bass_utils import run_bass_kernel_spmd

F32 = mybir.dt.float32
F32R = mybir.dt.float32r
BF16 = mybir.dt.bfloat16
FP8 = mybir.dt.float8e4

B, N, D = 2, 8192, 512
RPC = N // 4  # rows per core: 2048
INV_SCALE = float(1.0 / np.sqrt(np.float32(D)))

_CACHE = {}


def _build(stage="full"):
    # stage: debug gate — "p1" (projections only), "p2" (+w), "coll" (+AllGather),
    # "full" (everything). kernel() always uses "full".
    nc = bacc.Bacc(None, target_bir_lowering=False, debug=False, num_devices=8)

    adjt = nc.dram_tensor("adjt", [N, RPC], FP8, kind="ExternalInput")
    xt = nc.dram_tensor("xt", [D, N], BF16, kind="ExternalInput")
    xtq = nc.dram_tensor("xtq", [D, RPC], F32, kind="ExternalInput")
    xtqb = nc.dram_tensor("xtqb", [D, RPC], BF16, kind="ExternalInput")
    wqd = nc.dram_tensor("wq", [D, D], BF16, kind="ExternalInput")
    wkd = nc.dram_tensor("wk", [D, D], BF16, kind="ExternalInput")
    wvd = nc.dram_tensor("wv", [D, D], F32, kind="ExternalInput")
    maskqd = nc.dram_tensor("maskq", [128, 64], F32, kind="ExternalInput")
    maskownd = nc.dram_tensor("maskown", [128, 16], F32, kind="ExternalInput")
    bseld = nc.dram_tensor("bsel", [128, 2], F32, kind="ExternalInput")
    loutd = nc.dram_tensor("lout", [RPC, D], F32, kind="ExternalOutput")

    w_locds = [nc.dram_tensor(f"w_loc{qq}", [128, 4], F32) for qq in range(4)]
    w_allds = [nc.dram_tensor(f"w_all{qq}", [8, 128, 4], F32, addr_space="Shared")
               for qq in range(4)]
    dwarm_in = nc.dram_tensor("dwarm_in", [1, 16], F32)
    dwarm_out = nc.dram_tensor("dwarm_out", [8, 1, 16], F32, addr_space="Shared")

    MUL = mybir.AluOpType.mult
    ADD = mybir.AluOpType.add
    X_AX = mybir.AxisListType.X

    with tile.TileContext(nc) as tc, ExitStack() as ctx:
        wtail = ctx.enter_context(tc.tile_pool(name="wtail", bufs=1))
        bigctx = ExitStack()
        big = bigctx.enter_context(tc.tile_pool(name="big", bufs=1))

        km_s = big.tile([128, 64, D], FP8)       # k*mask, [j-part, j-chunk, d]
        q_s = big.tile([128, 16, D], F32)        # own q rows
        wq_s = big.tile([128, 4, D], BF16)
        wk_s = big.tile([128, 4, D], BF16)
        maskq_s = big.tile([128, 64], F32)
        wv_s = big.tile([128, 4, D], F32R)

        w_sb = wtail.tile([128, 16], F32)        # own w (pre-mask)
        w2 = wtail.tile([128, 16], F32)          # own w (masked)
        maskown_s = wtail.tile([128, 16], F32)
        v_sb = wtail.tile([128, 16, D], F32)     # own v rows (fp32r matmul)
        bsel_s = wtail.tile([128, 2], F32)

        for cc in range(4):
            nc.sync.dma_start(wk_s[:, cc, :], wkd[cc * 128:(cc + 1) * 128, :])
            nc.sync.dma_start(wq_s[:, cc, :], wqd[cc * 128:(cc + 1) * 128, :])
        nc.sync.dma_start(maskq_s[:], maskqd[:])
        nc.sync.dma_start(wv_s[:], wvd[:].bitcast(F32R).rearrange("(cc p) d -> p cc d", p=128))
        nc.sync.dma_start(maskown_s[:], maskownd[:])
        nc.sync.dma_start(bsel_s[:], bseld[:])

        # warmups, off the critical path: exp LUT load + collective firmware
        warm = wtail.tile([128, 16], F32)
        nc.vector.memset(warm[:], 0.0)
        nc.scalar.activation(out=warm[:], in_=warm[:],
                             func=mybir.ActivationFunctionType.Exp, bias=0.0, scale=1.0)
        nc.sync.dma_start(dwarm_in[:], warm[0:1, :])
        nc.gpsimd.collective_compute(
            "AllGather",
            mybir.AluOpType.bypass,
            replica_groups=[[0, 1, 2, 3, 4, 5, 6, 7]],
            ins=[dwarm_in[:]],
            outs=[dwarm_out[:]],
        )

        # ---- Phase 1: k_m (all N rows) and q (own rows) projections -------
        with (
            tc.tile_pool(name="p1", bufs=16) as p1pool,
            tc.tile_pool(name="ps1", bufs=6, space="PSUM") as ps1,
        ):
            for jp in range(16):  # panels of 512 bag rows
                xtiles = []
                for cc in range(4):
                    xt_t = p1pool.tile([128, 512], BF16, tag="xtp")
                    nc.sync.dma_start(
                        xt_t[:],
                        xt[cc * 128:(cc + 1) * 128, jp * 512:(jp + 1) * 512],
                    )
                    xtiles.append(xt_t)
                for jc2 in range(4):
                    jc = jp * 4 + jc2
                    ps_k = ps1.tile([128, D], F32, tag="psk")
                    for cc in range(4):
                        nc.tensor.matmul(
                            ps_k[:],
                            lhsT=xtiles[cc][:, jc2 * 128:(jc2 + 1) * 128],
                            rhs=wk_s[:, cc, :],
                            start=(cc == 0),
                            stop=(cc == 3),
                        )
                    nc.vector.tensor_scalar_mul(km_s[:, jc, :], ps_k[:], maskq_s[:, jc:jc + 1])
            for gp in range(4):  # own panels of 512 rows
                xqt = []
                for cc in range(4):
                    xq_t = p1pool.tile([128, 512], BF16, tag="xtp")
                    nc.sync.dma_start(
                        xq_t[:],
                        xtqb[cc * 128:(cc + 1) * 128, gp * 512:(gp + 1) * 512],
                    )
                    xqt.append(xq_t)
                xvt = []
                for cc in range(4):
                    xv_t = p1pool.tile([128, 512], F32R, tag="xvp")
                    nc.sync.dma_start(
                        xv_t[:],
                        xtq[cc * 128:(cc + 1) * 128, gp * 512:(gp + 1) * 512].bitcast(F32R),
                    )
                    xvt.append(xv_t)
                for t2 in range(4):
                    t = gp * 4 + t2
                    ps_q = ps1.tile([128, D], F32, tag="psk")
                    for cc in range(4):
                        nc.tensor.matmul(
                            ps_q[:],
                            lhsT=xqt[cc][:, t2 * 128:(t2 + 1) * 128],
                            rhs=wq_s[:, cc, :],
                            start=(cc == 0),
                            stop=(cc == 3),
                        )
                    nc.vector.tensor_copy(q_s[:, t, :], ps_q[:])
                for t2 in range(4):
                    t = gp * 4 + t2
                    ps_v = ps1.tile([128, D], F32, tag="psk")
                    for cc in range(4):
                        nc.tensor.matmul(
                            ps_v[:],
                            lhsT=xvt[cc][:, t2 * 128:(t2 + 1) * 128],
                            rhs=wv_s[:, cc, :],
                            start=(cc == 0),
                            stop=(cc == 3),
                        )
                    nc.vector.tensor_copy(v_sb[:, t, :], ps_v[:])

        if stage == "p1":
            with tc.tile_pool(name="dbg", bufs=2) as dbg:
                for t in range(16):
                    dt_ = dbg.tile([128, D], F32, tag="dbg")
                    nc.vector.tensor_copy(dt_[:], km_s[:, t, :])
                    nc.sync.dma_start(loutd[t * 128:(t + 1) * 128, :], dt_[:])
            bigctx.close()

        # ---- Phase 2: agg = adj_block @ k_m ; w = inv_scale * rowdot(q, agg)
        if stage != "p1":
            with (
                tc.tile_pool(name="s2", bufs=16) as s2pool,
                tc.tile_pool(name="scrp", bufs=4) as scrpool,
                tc.tile_pool(name="ps2", bufs=8, space="PSUM") as ps2,
            ):
                for qq in range(4):  # quarters of 512 own rows -> 4 PSUM banks
                    aggs = [ps2.tile([128, D], F32, tag="agg", name=f"agg_{qq}_{i}") for i in range(4)]
                    for jb in range(16):  # batches of 4 j-chunks (512 KB DMA)
                        at = s2pool.tile([128, 4, 512], FP8, tag="adjs")
                        nc.sync.dma_start(
                            at[:],
                            adjt[jb * 512:(jb + 1) * 512, qq * 512:(qq + 1) * 512]
                            .rearrange("(jc2 p) i -> p jc2 i", p=128),
                        )
                        for u in range(2):  # chunk pairs -> fp8 DoubleRow (K=256/MM)
                            jp2 = jb * 2 + u
                            for is_ in range(4):
                                nc.tensor.matmul(
                                    aggs[is_][:],
                                    lhsT=at[:, 2 * u:2 * u + 2, is_ * 128:(is_ + 1) * 128],
                                    rhs=km_s[:, 4 * jb + 2 * u:4 * jb + 2 * u + 2, :],
                                    start=(jp2 == 0),
                                    stop=(jp2 == 31),
                                    perf_mode=mybir.MatmulPerfMode.DoubleRow,
                                )
                    for is_ in range(4):
                        t = qq * 4 + is_
                        # NOTE: tensor_tensor_reduce with a PSUM in0 faults the
                        # device (HW-only, sim-clean) — use mul + reduce instead.
                        scr = scrpool.tile([128, D], F32, tag="scr")
                        nc.vector.tensor_mul(scr[:], aggs[is_][:], q_s[:, t, :])
                        nc.vector.reduce_sum(out=w_sb[:, t:t + 1], in_=scr[:], axis=X_AX)
                    # finalize + publish this quarter's w; gathers for quarters
                    # 0-2 hide under the remaining matmul stream
                    qsl = slice(qq * 4, qq * 4 + 4)
                    nc.vector.tensor_scalar_mul(w2[:, qsl], w_sb[:, qsl], INV_SCALE)
                    nc.vector.tensor_mul(w2[:, qsl], w2[:, qsl], maskown_s[:, qsl])
                    if stage != "p2":
                        nc.sync.dma_start(w_locds[qq][:], w2[:, qsl])
                        nc.gpsimd.collective_compute(
                            "AllGather",
                            mybir.AluOpType.bypass,
                            replica_groups=[[0, 1, 2, 3, 4, 5, 6, 7]],
                            ins=[w_locds[qq][:]],
                            outs=[w_allds[qq][:]],
                        )

            bigctx.close()  # frees km/q/wq/wk (128+32+16 KB/partition) for the tail
            if stage == "p2":
                nc.sync.dma_start(loutd[0:128, 0:16], w2[:])

        if stage == "coll":
            with tc.tile_pool(name="dbg2", bufs=1) as dbg2:
                wdbg = dbg2.tile([128, 128], F32)
                for qq in range(4):
                    nc.sync.dma_start(
                        wdbg[:].rearrange("p (g q t) -> p g q t", g=8, q=4)[:, :, qq, :],
                        w_allds[qq][:].rearrange("g p t -> p g t"),
                    )
                nc.sync.dma_start(loutd[0:128, 0:128], wdbg[:])

        if stage == "full":
            with (
                tc.tile_pool(name="tail", bufs=1) as tailp,
                tc.tile_pool(name="ltp", bufs=8) as ltp,
            ):
                # ---- Phase 4: softmax normalizers for both batches --------
                wall = tailp.tile([128, 128], F32)
                for qq in range(4):
                    nc.sync.dma_start(
                        wall[:].rearrange("p (g q t) -> p g q t", g=8, q=4)[:, :, qq, :],
                        w_allds[qq][:].rearrange("g p t -> p g t"),
                    )
                gnegs, rinvs = [], []
                for h in range(2):
                    wh = wall[:, h * 64:(h + 1) * 64]
                    m1 = tailp.tile([128, 1], F32, tag=f"m1_{h}", name=f"m1_{h}")
                    nc.vector.reduce_max(out=m1[:], in_=wh, axis=X_AX)
                    gmax = tailp.tile([128, 1], F32, tag=f"gmax_{h}", name=f"gmax_{h}")
                    nc.gpsimd.partition_all_reduce(
                        gmax[:], m1[:], channels=128, reduce_op=bass_isa.ReduceOp.max
                    )
                    gneg = tailp.tile([128, 1], F32, tag=f"gneg_{h}", name=f"gneg_{h}")
                    nc.vector.tensor_scalar_mul(gneg[:], gmax[:], -1.0)
                    eh = tailp.tile([128, 64], F32, tag=f"eh_{h}", name=f"eh_{h}")
                    nc.scalar.activation(
                        out=eh[:], in_=wh, func=mybir.ActivationFunctionType.Exp,
                        bias=gneg[:], scale=1.0,
                    )
                    s1 = tailp.tile([128, 1], F32, tag=f"s1_{h}", name=f"s1_{h}")
                    nc.vector.reduce_sum(out=s1[:], in_=eh[:], axis=X_AX)
                    gsum = tailp.tile([128, 1], F32, tag=f"gsum_{h}", name=f"gsum_{h}")
                    nc.gpsimd.partition_all_reduce(
                        gsum[:], s1[:], channels=128, reduce_op=bass_isa.ReduceOp.add
                    )
                    rinv = tailp.tile([128, 1], F32, tag=f"rinv_{h}", name=f"rinv_{h}")
                    nc.vector.reciprocal(rinv[:], gsum[:])
                    gnegs.append(gneg)
                    rinvs.append(rinv)

                # select my batch's normalizers via bsel (per-core input)
                ga = tailp.tile([128, 1], F32, tag="ga")
                gb = tailp.tile([128, 1], F32, tag="gb")
                gneg_my = tailp.tile([128, 1], F32, tag="gneg_my")
                nc.vector.tensor_mul(ga[:], gnegs[0][:], bsel_s[:, 0:1])
                nc.vector.tensor_mul(gb[:], gnegs[1][:], bsel_s[:, 1:2])
                nc.vector.tensor_add(gneg_my[:], ga[:], gb[:])
                ra = tailp.tile([128, 1], F32, tag="ra")
                rb = tailp.tile([128, 1], F32, tag="rb")
                rinv_my = tailp.tile([128, 1], F32, tag="rinv_my")
                nc.vector.tensor_mul(ra[:], rinvs[0][:], bsel_s[:, 0:1])
                nc.vector.tensor_mul(rb[:], rinvs[1][:], bsel_s[:, 1:2])
                nc.vector.tensor_add(rinv_my[:], ra[:], rb[:])

                eown = tailp.tile([128, 16], F32, tag="eown")
                nc.scalar.activation(
                    out=eown[:], in_=w2[:], func=mybir.ActivationFunctionType.Exp,
                    bias=gneg_my[:], scale=1.0,
                )
                pown = tailp.tile([128, 16], F32, tag="pown")
                nc.vector.tensor_scalar_mul(pown[:], eown[:], rinv_my[:])

                # ---- Phase 5b: L rows = p_i * v_i -------------------------
                for t in range(16):
                    lt = ltp.tile([128, D], F32, tag="lt")
                    nc.vector.tensor_scalar_mul(lt[:], v_sb[:, t, :], pown[:, t:t + 1])
                    eng = nc.sync if t % 2 == 0 else nc.gpsimd
                    eng.dma_start(loutd[t * 128:(t + 1) * 128, :], lt[:])

    nc.finalize()
    return nc


def _prep_inputs(X, adj, mask, Wqk, Wv):
    import ml_dtypes
    bf16 = ml_dtypes.bfloat16
    fp8 = ml_dtypes.float8_e4m3
    X = np.ascontiguousarray(np.asarray(X, dtype=np.float32))
    adj = np.asarray(adj, dtype=np.float32)
    mask = np.ascontiguousarray(np.asarray(mask, dtype=np.float32))
    Wqk = np.asarray(Wqk, dtype=np.float32)
    Wv = np.ascontiguousarray(np.asarray(Wv, dtype=np.float32))
    wq_h = np.ascontiguousarray(Wqk[:, :D].astype(bf16))
    wk_h = np.ascontiguousarray(Wqk[:, D:].astype(bf16))

    in_maps = []
    for b in range(B):
        xt_b = np.ascontiguousarray(X[b].T)
        xt_bh = np.ascontiguousarray(xt_b.astype(bf16))
        adjt_bh = np.ascontiguousarray(adj[b].astype(fp8).T)
        maskq_b = np.ascontiguousarray(mask[b].reshape(64, 128).T)
        for r in range(4):
            i0 = r * RPC
            bsel = np.zeros((128, 2), np.float32)
            bsel[:, b] = 1.0
            in_maps.append({
                "adjt": np.ascontiguousarray(adjt_bh[:, i0:i0 + RPC]),
                "xt": xt_bh,
                "xtq": np.ascontiguousarray(xt_b[:, i0:i0 + RPC]),
                "xtqb": np.ascontiguousarray(xt_bh[:, i0:i0 + RPC]),
                "wq": wq_h,
                "wk": wk_h,
                "wv": Wv,
                "maskq": maskq_b,
                "maskown": np.ascontiguousarray(mask[b, i0:i0 + RPC].reshape(16, 128).T),
                "bsel": bsel,
            })
    return in_maps


def _run(inputs, **kwargs):
    if "nc" not in _CACHE:
        _CACHE["nc"] = _build()
    nc = _CACHE["nc"]
    in_maps = _prep_inputs(**inputs)
    res = run_bass_kernel_spmd(nc, in_maps, list(range(8)), **kwargs)
    L = np.empty((B, N, D), np.float32)
    for c in range(8):
        b, r = divmod(c, 4)
        L[b, r * RPC:(r + 1) * RPC] = res.results[c]["lout"]
    return L, res


def kernel(X, adj, mask, Wqk, Wv):
    L, _ = _run(dict(X=X, adj=adj, mask=mask, Wqk=Wqk, Wv=Wv))
    return L
